# revision 1
# baseline (speedup 1.0000x reference)
# Trainium2 Bass kernel for nn_ModelPositional (gnn_message_passing).
#
# Reference computation (B=4, S=512, K=16, V=50265, D=768, DC=784):
#   nodes = pos==0; token = pos>=2
#   emb = emb_table[code]                                  [B,S,768]
#   ntm = nodes[:,None]&token[None,:]&attn  (row-normalized)
#   emb = where(nodes, ntm@emb, emb)
#   pe  = diag(M^t) t=1..16, M = D^-1 A, A = attn|I        [B,S,16]
#   x   = [emb | pe]                                       [B,S,784]
#   out = x @ w_lin + b_lin                                [B,S,50265]
#
# Sharding (8 cores): core c = (batch b=c//2, row-half h=c%2).
#  Stage 1 (per core): RWPE chain for its 256 rows + masked avg + gather,
#    building xT_local [896, 256] (d-major, bias ones-row at d=784).
#    Host permutes token order per core so its rows are always local 0..255
#    (keeps the SPMD program identical across cores).
#  AllGather xT slices -> xT [896, 2048] on every core.
#  Stage 2: logits[:, c*6284:(c+1)*6284] = xT.T @ w_pad_c  (vocab-sharded).
# Host concatenates the 8 vocab slices.
#
# All matmuls run as float32r (full-rate on PE; fp32 would be 4x slower).

import numpy as np

B, S, KPE, V, D = 4, 512, 16, 50265, 768
NCORES = 8
VPAD = 50272          # 8 * 6284
VC = VPAD // NCORES   # 6284 vocab columns per core
DPAD = 896            # 7 * 128 (784 true dims + bias row at 784 + zero pad)
KCH = DPAD // 128     # 7 contraction chunks
P = 128
NTOK = B * S          # 2048
MT = NTOK // P        # 16 token tiles

_CACHE = {}


def _build_nc(vc=VC):
    import concourse.bacc as bacc
    import concourse.bass as bass
    import concourse.mybir as mybir
    import concourse.tile as tile
    from concourse.bass import IndirectOffsetOnAxis
    from concourse.masks import make_identity

    f32 = mybir.dt.float32
    f32r = mybir.dt.float32r
    i32 = mybir.dt.int32
    Alu = mybir.AluOpType
    AX = mybir.AxisListType

    nc = bacc.Bacc(
        "TRN2",
        target_bir_lowering=False,
        debug=False,
        num_devices=NCORES,
    )

    attn_in = nc.dram_tensor("attn", [S, S], f32, kind="ExternalInput").ap()
    attnT_in = nc.dram_tensor("attnT", [S, 256], f32, kind="ExternalInput").ap()
    codes_in = nc.dram_tensor("codes", [P, 4], i32, kind="ExternalInput").ap()
    token_in = nc.dram_tensor("token_ps", [P, 4], f32, kind="ExternalInput").ap()
    nodes_in = nc.dram_tensor("nodes_ps", [P, 2], f32, kind="ExternalInput").ap()
    emb_in = nc.dram_tensor("emb_table", [V, D], f32, kind="ExternalInput").ap()
    w_in = nc.dram_tensor("w_pad", [DPAD, vc], f32r, kind="ExternalInput").ap()
    logit_out = nc.dram_tensor("logit", [NTOK, vc], f32, kind="ExternalOutput").ap()

    n_full, n_rem = divmod(vc, 512)
    ntiles = [512] * n_full + ([n_rem] if n_rem else [])

    with tile.TileContext(nc) as tc:
        with (
            tc.tile_pool(name="persist", bufs=1) as persist,
            tc.tile_pool(name="dram", bufs=1, space="DRAM") as dram,
        ):
            # xT for ALL tokens (post-AllGather): [128, 8cores*7chunks, 256]
            xT_all = persist.tile([P, NCORES * KCH, 256], f32r, name="xT_all")
            cc_in = dram.tile([DPAD, 256], f32, name="cc_in")
            cc_out = dram.tile(
                [NCORES * DPAD, 256], f32, name="cc_out", addr_space="Shared"
            )

            # ---------------- Stage 1 ----------------
            with (
                tc.tile_pool(name="s1", bufs=1) as s1,
                tc.tile_pool(name="s1tmp", bufs=3) as s1t,
                tc.tile_pool(name="psq", bufs=3, space="PSUM") as psq,
                tc.tile_pool(name="psa", bufs=2, space="PSUM") as psa,
                tc.tile_pool(name="pst", bufs=2, space="PSUM") as pst,
            ):
                ident = s1.tile([P, P], f32, name="ident")
                make_identity(nc, ident[:])

                attn_sb = s1.tile([P, 4, S], f32, name="attn_sb")
                nc.sync.dma_start(
                    out=attn_sb[:], in_=attn_in.rearrange("(j p) s -> p j s", p=P)
                )
                attnT_sb = s1.tile([P, 4, 256], f32, name="attnT_sb")
                nc.sync.dma_start(
                    out=attnT_sb[:], in_=attnT_in.rearrange("(k p) r -> p k r", p=P)
                )
                codes_sb = s1.tile([P, 4], i32, name="codes_sb")
                nc.sync.dma_start(out=codes_sb[:], in_=codes_in)
                token_sb = s1.tile([P, 4], f32, name="token_sb")
                nc.sync.dma_start(out=token_sb[:], in_=token_in)
                nodes_sb = s1.tile([P, 2], f32, name="nodes_sb")
                nc.sync.dma_start(out=nodes_sb[:], in_=nodes_in)

                # eye blocks: eye4[:, j, :] has 1 at (p, j*128+p)
                eye4 = s1.tile([P, 4, S], f32, name="eye4")
                nc.gpsimd.memset(eye4[:], 0.0)
                for j in range(4):
                    nc.gpsimd.affine_select(
                        out=eye4[:, j, :],
                        in_=eye4[:, j, :],
                        compare_op=Alu.not_equal,
                        fill=1.0,
                        base=j * P,
                        pattern=[[-1, S]],
                        channel_multiplier=1,
                    )

                # A = max(attn, I); deg = rowsum(A); M = A / deg
                m_sb = s1.tile([P, 4, S], f32r, name="m_sb")
                for j in range(4):
                    a_j = s1t.tile([P, S], f32, tag="a_j")
                    nc.vector.tensor_tensor(
                        out=a_j[:], in0=attn_sb[:, j, :], in1=eye4[:, j, :], op=Alu.max
                    )
                    deg_j = s1t.tile([P, 1], f32, tag="deg_j")
                    nc.vector.reduce_sum(out=deg_j[:], in_=a_j[:], axis=AX.X)
                    rinv_j = s1t.tile([P, 1], f32, tag="rinv_j")
                    nc.vector.reciprocal(out=rinv_j[:], in_=deg_j[:])
                    nc.vector.tensor_scalar_mul(m_sb[:, j, :], a_j[:], rinv_j[:])

                # Q0 = I[:, 0:256] (our rows are local 0..255)
                qA = s1.tile([P, 4, 256], f32r, name="qA")
                qB = s1.tile([P, 4, 256], f32r, name="qB")
                q0f = s1.tile([P, 4, 256], f32, name="q0f")
                nc.gpsimd.memset(q0f[:], 0.0)
                for jb in range(2):
                    nc.gpsimd.affine_select(
                        out=q0f[:, jb, :],
                        in_=q0f[:, jb, :],
                        compare_op=Alu.not_equal,
                        fill=1.0,
                        base=jb * P,
                        pattern=[[-1, 256]],
                        channel_multiplier=1,
                    )
                nc.vector.tensor_copy(out=qA[:], in_=q0f[:])

                # 128 columns: 16 pe steps, a ones column at 16 (becomes the
                # bias row at d=784 after transposition), zeros beyond.
                pe_pad = s1.tile([P, 2, P], f32, name="pe_pad")
                nc.gpsimd.memset(pe_pad[:], 0.0)
                nc.vector.memset(pe_pad[:, :, KPE : KPE + 1], 1.0)

                # chain: Q_{t+1} = M^T @ Q_t ; diag(M^{t+1})[r] = Q_{t+1}[r, r]
                cur = qA
                for t in range(KPE):
                    nxt = qB if cur is qA else qA
                    for i in range(4):
                        pq = psq.tile([P, 256], f32, tag="pq")
                        for j in range(4):
                            nc.tensor.matmul(
                                out=pq[:],
                                lhsT=m_sb[:, j, i * P : (i + 1) * P],
                                rhs=cur[:, j, :],
                                start=(j == 0),
                                stop=(j == 3),
                            )
                        if i < 2:
                            # diag of this 128-row block: E mask = eye4 slice
                            # (tensor_tensor_reduce crashes on HW; use mul+red)
                            dummy = s1t.tile([P, 256], f32, tag="ttr_dummy")
                            nc.vector.tensor_mul(dummy[:], pq[:], eye4[:, i, 0:256])
                            nc.vector.reduce_sum(
                                out=pe_pad[:, i, t : t + 1], in_=dummy[:], axis=AX.X
                            )
                        nc.vector.tensor_copy(out=nxt[:, i, :], in_=pq[:])
                    cur = nxt

                # ntm^T = attnT * token (per-partition scalar over s)
                ntmT = s1.tile([P, 4, 256], f32r, name="ntmT")
                for k in range(4):
                    nc.vector.tensor_scalar_mul(
                        ntmT[:, k, :], attnT_sb[:, k, :], token_sb[:, k : k + 1]
                    )

                # gather embeddings for all 512 tokens of this batch (+ones col)
                emb_plus = s1.tile([P, 4, D + 2], f32, name="emb_plus")
                for k in range(4):
                    nc.gpsimd.indirect_dma_start(
                        out=emb_plus[:, k, 0:D],
                        out_offset=None,
                        in_=emb_in[:, :],
                        in_offset=IndirectOffsetOnAxis(
                            ap=codes_sb[:, k : k + 1], axis=0
                        ),
                    )
                nc.vector.memset(emb_plus[:, :, D : D + 1], 1.0)
                nc.vector.memset(emb_plus[:, :, D + 1 : D + 2], 0.0)
                emb_b = s1.tile([P, 4, D + 2], f32r, name="emb_b")
                nc.vector.tensor_copy(out=emb_b[:], in_=emb_plus[:])

                # avg rows (for our 256 rows) + row-sum via the ones column
                embo = s1.tile([P, 2, D], f32, name="embo")
                for lj in range(2):
                    pav0 = psa.tile([P, 512], f32, tag="pav")
                    pav1 = psa.tile([P, 512], f32, tag="pav")
                    for k in range(4):
                        nc.tensor.matmul(
                            out=pav0[:],
                            lhsT=ntmT[:, k, lj * P : (lj + 1) * P],
                            rhs=emb_b[:, k, 0:512],
                            start=(k == 0),
                            stop=(k == 3),
                        )
                    for k in range(4):
                        nc.tensor.matmul(
                            out=pav1[:, 0 : D + 2 - 512],
                            lhsT=ntmT[:, k, lj * P : (lj + 1) * P],
                            rhs=emb_b[:, k, 512 : D + 2],
                            start=(k == 0),
                            stop=(k == 3),
                        )
                    rs = s1t.tile([P, 1], f32, tag="rs")
                    nc.vector.tensor_scalar_add(rs[:], pav1[:, D - 512 : D - 511], 1e-10)
                    rinv = s1t.tile([P, 1], f32, tag="rinv")
                    nc.vector.reciprocal(out=rinv[:], in_=rs[:])
                    avg_n = s1t.tile([P, D], f32, tag="avg_n")
                    nc.vector.tensor_scalar_mul(avg_n[:, 0:512], pav0[:], rinv[:])
                    nc.vector.tensor_scalar_mul(
                        avg_n[:, 512:D], pav1[:, 0 : D - 512], rinv[:]
                    )
                    # emb_out = emb + nodes * (avg - emb)
                    d1 = s1t.tile([P, D], f32, tag="d1")
                    nc.vector.tensor_sub(d1[:], avg_n[:], emb_plus[:, lj, 0:D])
                    nc.vector.tensor_scalar_mul(d1[:], d1[:], nodes_sb[:, lj : lj + 1])
                    nc.vector.tensor_add(embo[:, lj, :], emb_plus[:, lj, 0:D], d1[:])

                # assemble xT_local [128, 7, 256] (transpose embo; append pe,
                # ones row at d=784, zeros beyond)
                xT_loc = s1.tile([P, KCH, 256], f32r, name="xT_loc")
                for lj in range(2):
                    for kk in range(6):
                        pt = pst.tile([P, P], f32, tag="pt")
                        nc.tensor.transpose(
                            out=pt[:],
                            in_=embo[:, lj, kk * P : (kk + 1) * P],
                            identity=ident[:],
                        )
                        nc.vector.tensor_copy(
                            out=xT_loc[:, kk, lj * P : (lj + 1) * P], in_=pt[:]
                        )
                    pt2 = pst.tile([P, P], f32, tag="pt")
                    nc.tensor.transpose(
                        out=pt2[:], in_=pe_pad[:, lj, :], identity=ident[:]
                    )
                    nc.vector.tensor_copy(
                        out=xT_loc[:, KCH - 1, lj * P : (lj + 1) * P],
                        in_=pt2[:],
                    )

                nc.sync.dma_start(
                    out=cc_in.rearrange("(k p) r -> p k r", p=P),
                    in_=xT_loc[:].bitcast(f32),
                )

            # ---------------- AllGather ----------------
            nc.gpsimd.collective_compute(
                "AllGather",
                mybir.AluOpType.bypass,
                replica_groups=[list(range(NCORES))],
                ins=[cc_in[:].opt()],
                outs=[cc_out[:].opt()],
            )
            nc.sync.dma_start(
                out=xT_all[:],
                in_=cc_out.rearrange("(ck p) r -> p ck r", p=P).bitcast(f32r),
            )

            # ---------------- Stage 2: logits = xT.T @ w ----------------
            with (
                tc.tile_pool(name="wp", bufs=3) as wp,
                tc.tile_pool(name="ob", bufs=4) as obp,
                tc.tile_pool(name="ps2", bufs=4, space="PSUM") as ps2,
            ):
                w_re = w_in.rearrange("(k p) v -> p k v", p=P)
                for n, ntn in enumerate(ntiles):
                    n0 = n * 512
                    w_sb = wp.tile([P, KCH, 512], f32r, tag="w_sb")
                    nc.sync.dma_start(
                        out=w_sb[:, :, 0:ntn], in_=w_re[:, :, n0 : n0 + ntn]
                    )
                    for m in range(MT):
                        rc, lj = divmod(m, 2)
                        po = ps2.tile([P, 512], f32, tag="po")
                        for k in range(KCH):
                            nc.tensor.matmul(
                                out=po[:, 0:ntn],
                                lhsT=xT_all[
                                    :, rc * KCH + k, lj * P : (lj + 1) * P
                                ],
                                rhs=w_sb[:, k, 0:ntn],
                                start=(k == 0),
                                stop=(k == KCH - 1),
                            )
                        ob = obp.tile([P, 512], f32, tag="ob")
                        if m % 2 == 0:
                            nc.scalar.copy(out=ob[:, 0:ntn], in_=po[:, 0:ntn])
                        else:
                            nc.vector.tensor_copy(out=ob[:, 0:ntn], in_=po[:, 0:ntn])
                        nc.sync.dma_start(
                            out=logit_out[m * P : (m + 1) * P, n0 : n0 + ntn],
                            in_=ob[:, 0:ntn],
                        )

    nc.compile()
    return nc


def _host_prep(code_inputs, position_idx, attn_mask, emb_table, w_lin, b_lin, vc=VC):
    code = np.asarray(code_inputs).astype(np.int32)
    pos = np.asarray(position_idx).astype(np.int32)
    attn = np.asarray(attn_mask).astype(np.float32)
    emb_t = np.ascontiguousarray(np.asarray(emb_table, dtype=np.float32))
    w = np.asarray(w_lin, dtype=np.float32)
    bias = np.asarray(b_lin, dtype=np.float32)

    w_ext = np.zeros((DPAD, NCORES * vc), np.float32)
    ncols = min(NCORES * vc, V)
    w_ext[: D + KPE, :ncols] = w[:, :ncols]
    w_ext[D + KPE, :ncols] = bias[:ncols]

    nodes = (pos == 0).astype(np.float32)
    token = (pos >= 2).astype(np.float32)

    in_maps = []
    for c in range(NCORES):
        b, h = divmod(c, 2)
        if h == 0:
            perm = np.arange(S)
        else:
            perm = np.r_[256:512, 0:256]
        a_p = attn[b][perm][:, perm]
        in_maps.append(
            {
                "attn": np.ascontiguousarray(a_p),
                "attnT": np.ascontiguousarray(a_p[:256, :].T),
                "codes": np.ascontiguousarray(code[b][perm].reshape(4, P).T),
                "token_ps": np.ascontiguousarray(token[b][perm].reshape(4, P).T),
                "nodes_ps": np.ascontiguousarray(
                    nodes[b][perm][:256].reshape(2, P).T
                ),
                "emb_table": emb_t,
                "w_pad": np.ascontiguousarray(w_ext[:, c * vc : (c + 1) * vc]),
            }
        )
    return in_maps


def run(inputs, trace=False, vc=VC, **run_kwargs):
    from concourse.bass_utils import run_bass_kernel_spmd

    key = ("nc", vc)
    nc = _CACHE.get(key)
    if nc is None:
        nc = _build_nc(vc=vc)
        _CACHE[key] = nc
    in_maps = _host_prep(**inputs, vc=vc)
    res = run_bass_kernel_spmd(
        nc, in_maps, core_ids=list(range(NCORES)), trace=trace, **run_kwargs
    )
    ncols = min(NCORES * vc, V)
    logits = np.concatenate([r["logit"] for r in res.results], axis=1)[:, :ncols]
    return logits.reshape(B, S, ncols).astype(np.float32), res


def kernel(**inputs):
    logits, _ = run(inputs, trace=False)
    return logits



# revision 2
# speedup vs baseline: 1.1288x; 1.1288x over previous
# Trainium2 Bass kernel for nn_ModelPositional (gnn_message_passing).
#
# Reference computation (B=4, S=512, K=16, V=50265, D=768, DC=784):
#   nodes = pos==0; token = pos>=2
#   emb = emb_table[code]                                  [B,S,768]
#   ntm = nodes[:,None]&token[None,:]&attn  (row-normalized)
#   emb = where(nodes, ntm@emb, emb)
#   pe  = diag(M^t) t=1..16, M = D^-1 A, A = attn|I        [B,S,16]
#   x   = [emb | pe]                                       [B,S,784]
#   out = x @ w_lin + b_lin                                [B,S,50265]
#
# Sharding (8 cores): core c = (batch b=c//2, row-half h=c%2).
#  Host precomputes (mask arithmetic only): the RW transition matrix M,
#  and the fused node-token mixing matrix ntmT' (normalized, with the
#  identity blended in for non-node rows), so on-chip stage 1 is:
#    xT_emb chunks 0..5 = emb_bf16^T @ ntmT'   (direct d-major output)
#    RWPE chain in f32r; pe+bias -> chunk 6
#  Two AllGathers (bf16): emb chunks right after the avg block (hidden
#  under the RWPE chain), pe chunk after the chain.
#  Stage 2: w (bf16) fully resident in SBUF; m-outer/n-inner; logits
#  written bf16 (host converts to f32).
#
# Stage-2 matmuls in bf16 get the compiler-automatic fast weight load
# (FWL); f32r weights cannot, costing ~128 cycles per matmul.

import numpy as np

B, S, KPE, V, D = 4, 512, 16, 50265, 768
NCORES = 8
VPAD = 50272          # 8 * 6284
VC = VPAD // NCORES   # 6284 vocab columns per core
DPAD = 896            # 7 * 128 (784 true dims + bias row at 784 + zero pad)
KCH = DPAD // 128     # 7 contraction chunks
P = 128
NTOK = B * S          # 2048
MT = NTOK // P        # 16 token tiles

_CACHE = {}


def _build_nc(vc=VC):
    import concourse.bacc as bacc
    import concourse.bass as bass
    import concourse.mybir as mybir
    import concourse.tile as tile
    from concourse.bass import IndirectOffsetOnAxis
    from concourse.masks import make_identity

    f32 = mybir.dt.float32
    f32r = mybir.dt.float32r
    bf16 = mybir.dt.bfloat16
    i32 = mybir.dt.int32
    Alu = mybir.AluOpType
    AX = mybir.AxisListType

    nc = bacc.Bacc(
        "TRN2",
        target_bir_lowering=False,
        debug=False,
        num_devices=NCORES,
    )

    m_in = nc.dram_tensor("m_rw", [S, S], f32r, kind="ExternalInput").ap()
    ntmT_in = nc.dram_tensor("ntmT", [S, 256], bf16, kind="ExternalInput").ap()
    codes_in = nc.dram_tensor("codes", [P, 4], i32, kind="ExternalInput").ap()
    emb_in = nc.dram_tensor("emb_table", [V, D], f32, kind="ExternalInput").ap()
    w_in = nc.dram_tensor("w_pad", [DPAD, vc], bf16, kind="ExternalInput").ap()
    logit_out = nc.dram_tensor("logit", [NTOK, vc], bf16, kind="ExternalOutput").ap()

    n_full, n_rem = divmod(vc, 512)
    ntiles = [512] * n_full + ([n_rem] if n_rem else [])

    with tile.TileContext(nc) as tc:
        with (
            tc.tile_pool(name="persist", bufs=1) as persist,
            tc.tile_pool(name="dram", bufs=1, space="DRAM") as dram,
        ):
            # gathered xT: emb chunks [128, 8c*6k, 256], pe chunks [128, 8c, 256]
            xTg_a = persist.tile([P, NCORES * 6, 256], bf16, name="xTg_a")
            xTg_b = persist.tile([P, NCORES, 256], bf16, name="xTg_b")
            xT_a = persist.tile([P, 6, 256], bf16, name="xT_a")
            xT_b = persist.tile([P, 1, 256], bf16, name="xT_b")
            w_all = persist.tile([P, KCH, vc], bf16, name="w_all")
            ident = persist.tile([P, P], f32, name="ident")

            cc_in_a = dram.tile([6 * P, 256], bf16, name="cc_in_a")
            cc_out_a = dram.tile(
                [NCORES * 6 * P, 256], bf16, name="cc_out_a", addr_space="Shared"
            )
            cc_in_b = dram.tile([P, 256], bf16, name="cc_in_b")
            cc_out_b = dram.tile(
                [NCORES * P, 256], bf16, name="cc_out_b", addr_space="Shared"
            )

            # w resident load: one DMA per contraction chunk (12.5KB/row each)
            w_re = w_in.rearrange("(k p) v -> p k v", p=P)
            for k in range(KCH):
                nc.sync.dma_start(out=w_all[:, k, :], in_=w_re[:, k, :])

            # ---------------- Stage 1 ----------------
            with (
                tc.tile_pool(name="s1", bufs=1) as s1,
                tc.tile_pool(name="s1tmp", bufs=3) as s1t,
                tc.tile_pool(name="psq", bufs=3, space="PSUM") as psq,
                tc.tile_pool(name="psa", bufs=2, space="PSUM") as psa,
                tc.tile_pool(name="pst", bufs=2, space="PSUM") as pst,
            ):
                make_identity(nc, ident[:])

                m_sb = s1.tile([P, 4, S], f32r, name="m_sb")
                nc.sync.dma_start(
                    out=m_sb[:], in_=m_in.rearrange("(j p) s -> p j s", p=P)
                )
                ntmT_sb = s1.tile([P, 4, 256], bf16, name="ntmT_sb")
                nc.sync.dma_start(
                    out=ntmT_sb[:], in_=ntmT_in.rearrange("(j p) r -> p j r", p=P)
                )
                codes_sb = s1.tile([P, 4], i32, name="codes_sb")
                nc.sync.dma_start(out=codes_sb[:], in_=codes_in)

                # eye blocks for Q0 init + diag extraction mask
                q0f = s1.tile([P, 4, 256], f32, name="q0f")
                nc.gpsimd.memset(q0f[:], 0.0)
                for jb in range(2):
                    nc.gpsimd.affine_select(
                        out=q0f[:, jb, :],
                        in_=q0f[:, jb, :],
                        compare_op=Alu.not_equal,
                        fill=1.0,
                        base=jb * P,
                        pattern=[[-1, 256]],
                        channel_multiplier=1,
                    )

                # 128 columns: 16 pe steps, ones column at 16 (bias row after
                # transposition), zeros beyond.
                pe_pad = s1.tile([P, 2, P], f32, name="pe_pad")
                nc.gpsimd.memset(pe_pad[:], 0.0)
                nc.vector.memset(pe_pad[:, :, KPE : KPE + 1], 1.0)

                # gather embeddings for all 512 tokens of this core's batch
                emb_sb = s1.tile([P, 4, D], f32, name="emb_sb")
                for k in range(4):
                    nc.gpsimd.indirect_dma_start(
                        out=emb_sb[:, k, :],
                        out_offset=None,
                        in_=emb_in[:, :],
                        in_offset=IndirectOffsetOnAxis(
                            ap=codes_sb[:, k : k + 1], axis=0
                        ),
                    )
                emb_bf = s1.tile([P, 4, D], bf16, name="emb_bf")
                for k in range(4):
                    if k % 2 == 0:
                        nc.scalar.copy(out=emb_bf[:, k, :], in_=emb_sb[:, k, :])
                    else:
                        nc.vector.tensor_copy(out=emb_bf[:, k, :], in_=emb_sb[:, k, :])

                # xT emb chunks: xT[c*128+d, r] = sum_s emb[s, c*128+d]*ntmT'[s, r]
                for c in range(6):
                    pa = psa.tile([P, 256], f32, tag="pa")
                    for j in range(4):
                        nc.tensor.matmul(
                            out=pa[:],
                            lhsT=emb_bf[:, j, c * P : (c + 1) * P],
                            rhs=ntmT_sb[:, j, :],
                            start=(j == 0),
                            stop=(j == 3),
                        )
                    if c % 2 == 0:
                        nc.scalar.copy(out=xT_a[:, c, :], in_=pa[:])
                    else:
                        nc.vector.tensor_copy(out=xT_a[:, c, :], in_=pa[:])

                nc.sync.dma_start(
                    out=cc_in_a.rearrange("(k p) r -> p k r", p=P), in_=xT_a[:]
                )

                # ---- AllGather A (emb chunks), overlapped with RWPE below ----
                nc.gpsimd.collective_compute(
                    "AllGather",
                    mybir.AluOpType.bypass,
                    replica_groups=[list(range(NCORES))],
                    ins=[cc_in_a[:].opt()],
                    outs=[cc_out_a[:].opt()],
                )
                nc.sync.dma_start(
                    out=xTg_a[:],
                    in_=cc_out_a.rearrange("(ck p) r -> p ck r", p=P),
                )

                # ---- RWPE chain (f32r) ----
                qA = s1.tile([P, 4, 256], f32r, name="qA")
                qB = s1.tile([P, 4, 256], f32r, name="qB")
                nc.vector.tensor_copy(out=qA[:], in_=q0f[:])

                cur = qA
                for t in range(KPE):
                    nxt = qB if cur is qA else qA
                    for i in range(4):
                        pq = psq.tile([P, 256], f32, tag="pq")
                        for j in range(4):
                            nc.tensor.matmul(
                                out=pq[:],
                                lhsT=m_sb[:, j, i * P : (i + 1) * P],
                                rhs=cur[:, j, :],
                                start=(j == 0),
                                stop=(j == 3),
                            )
                        if i < 2:
                            # diag of this 128-row block via eye-mask + reduce
                            dummy = s1t.tile([P, 256], f32, tag="ttr_dummy")
                            nc.vector.tensor_mul(dummy[:], pq[:], q0f[:, i, 0:256])
                            nc.vector.reduce_sum(
                                out=pe_pad[:, i, t : t + 1], in_=dummy[:], axis=AX.X
                            )
                        nc.vector.tensor_copy(out=nxt[:, i, :], in_=pq[:])
                    cur = nxt

                # pe chunk: transpose [tokens, pe] -> [pe, tokens]
                for lj in range(2):
                    pt = pst.tile([P, P], f32, tag="pt")
                    nc.tensor.transpose(
                        out=pt[:], in_=pe_pad[:, lj, :], identity=ident[:]
                    )
                    nc.vector.tensor_copy(
                        out=xT_b[:, 0, lj * P : (lj + 1) * P], in_=pt[:]
                    )

                nc.sync.dma_start(out=cc_in_b[:, :], in_=xT_b[:, 0, :])

                # ---- AllGather B (pe chunk) ----
                nc.gpsimd.collective_compute(
                    "AllGather",
                    mybir.AluOpType.bypass,
                    replica_groups=[list(range(NCORES))],
                    ins=[cc_in_b[:].opt()],
                    outs=[cc_out_b[:].opt()],
                )
                nc.sync.dma_start(
                    out=xTg_b[:],
                    in_=cc_out_b.rearrange("(c p) r -> p c r", p=P),
                )

            # ---------------- Stage 2: logits = xT.T @ w ----------------
            with (
                tc.tile_pool(name="ob", bufs=2) as obp,
                tc.tile_pool(name="ps2", bufs=4, space="PSUM") as ps2,
            ):
                half = (n_full // 2) * 512
                for m in range(MT):
                    rc, lj = divmod(m, 2)
                    ob = obp.tile([P, vc], bf16, tag="ob")
                    for n, ntn in enumerate(ntiles):
                        n0 = n * 512
                        po = ps2.tile([P, 512], f32, tag="po")
                        for k in range(KCH):
                            if k < 6:
                                lhsT = xTg_a[:, rc * 6 + k, lj * P : (lj + 1) * P]
                            else:
                                lhsT = xTg_b[:, rc, lj * P : (lj + 1) * P]
                            nc.tensor.matmul(
                                out=po[:, 0:ntn],
                                lhsT=lhsT,
                                rhs=w_all[:, k, n0 : n0 + ntn],
                                start=(k == 0),
                                stop=(k == KCH - 1),
                            )
                        if n % 2 == 0:
                            nc.scalar.copy(out=ob[:, n0 : n0 + ntn], in_=po[:, 0:ntn])
                        else:
                            nc.vector.tensor_copy(
                                out=ob[:, n0 : n0 + ntn], in_=po[:, 0:ntn]
                            )
                        if n0 + ntn == half:
                            nc.sync.dma_start(
                                out=logit_out[m * P : (m + 1) * P, 0:half],
                                in_=ob[:, 0:half],
                            )
                    nc.sync.dma_start(
                        out=logit_out[m * P : (m + 1) * P, half:vc],
                        in_=ob[:, half:vc],
                    )

    nc.compile()
    return nc


def _host_prep(code_inputs, position_idx, attn_mask, emb_table, w_lin, b_lin, vc=VC):
    import ml_dtypes

    bf = ml_dtypes.bfloat16
    code = np.asarray(code_inputs).astype(np.int32)
    pos = np.asarray(position_idx).astype(np.int32)
    attn = np.asarray(attn_mask).astype(np.float32)
    emb_t = np.ascontiguousarray(np.asarray(emb_table, dtype=np.float32))
    w = np.asarray(w_lin, dtype=np.float32)
    bias = np.asarray(b_lin, dtype=np.float32)

    w_ext = np.zeros((DPAD, NCORES * vc), np.float32)
    ncols = min(NCORES * vc, V)
    w_ext[: D + KPE, :ncols] = w[:, :ncols]
    w_ext[D + KPE, :ncols] = bias[:ncols]
    w_ext = w_ext.astype(bf)

    nodes = (pos == 0).astype(np.float32)
    token = (pos >= 2).astype(np.float32)
    eye = np.eye(S, dtype=bool)

    in_maps = []
    for c in range(NCORES):
        b, h = divmod(c, 2)
        if h == 0:
            perm = np.arange(S)
        else:
            perm = np.r_[256:512, 0:256]
        a_p = attn[b][perm][:, perm]
        tok_p = token[b][perm]
        nod_p = nodes[b][perm]

        # RW transition matrix M = D^-1 (attn | I)
        A = np.where(eye, 1.0, a_p).astype(np.float32)
        m_rw = A / A.sum(1)[:, None]

        # fused mixing matrix (columns = this core's 256 rows):
        #   ntmT'[s,r] = alpha_r * token_s * attn[r,s] + (1-nodes_r)*delta[s==r]
        rowsum = (a_p[:256] * tok_p[None, :]).sum(1)
        alpha = nod_p[:256] / (rowsum + 1e-10)
        ntmT = a_p[:256].T * tok_p[:, None] * alpha[None, :]
        ntmT[:256][np.eye(256, dtype=bool)] += 1.0 - nod_p[:256]

        in_maps.append(
            {
                "m_rw": np.ascontiguousarray(m_rw),
                "ntmT": np.ascontiguousarray(ntmT.astype(bf)),
                "codes": np.ascontiguousarray(code[b][perm].reshape(4, P).T),
                "emb_table": emb_t,
                "w_pad": np.ascontiguousarray(w_ext[:, c * vc : (c + 1) * vc]),
            }
        )
    return in_maps


def run(inputs, trace=False, vc=VC, **run_kwargs):
    from concourse.bass_utils import run_bass_kernel_spmd

    key = ("nc", vc)
    nc = _CACHE.get(key)
    if nc is None:
        nc = _build_nc(vc=vc)
        _CACHE[key] = nc
    in_maps = _host_prep(**inputs, vc=vc)
    res = run_bass_kernel_spmd(
        nc, in_maps, core_ids=list(range(NCORES)), trace=trace, **run_kwargs
    )
    ncols = min(NCORES * vc, V)
    logits = np.concatenate(
        [r["logit"].astype(np.float32) for r in res.results], axis=1
    )[:, :ncols]
    return logits.reshape(B, S, ncols).astype(np.float32), res


def kernel(**inputs):
    logits, _ = run(inputs, trace=False)
    return logits
